# revision 1
# baseline (speedup 1.0000x reference)
"""GQA kernel v2 for Trainium2 (Bass/Tile), 8 NeuronCores.

Sharding: core c -> batch b=c//4, kv-head pair j=c%4 (kv heads 2j,2j+1,
q heads 8j..8j+7).  Each core computes out[b, :, 512j:512(j+1)] (pair-major
column order, untangled on host).

Design:
  - Priority column-chunked input DMA: wk/cos/sin/wq0 + x cols [0,1024)
    land first so attention starts ~20us in, the rest streams behind.
  - Heads processed in PAIRS (m, m+4): head m on partitions 0-63 of
    qts[m]/kt, head m+4 on 64-127.  Their K=64 S^T matmuls are emitted
    adjacently so the PE row-tiles them concurrently (2x S throughput).
  - exp split per band: head A via ACT (exact), head B via DVE int16
    Schraudolph trick writing bf16 bit patterns directly.
  - q-block attention (blocks of 1024 q cols): S-phase exps all causal
    bands (P^T persists for the block), then a PV-phase accumulates each
    (head, q-tile) output in a PSUM region [128,65] (ones col = softmax
    denominator) with CONSECUTIVE matmul groups (PSUM allows only one
    open accumulation group per bank).
  - All remaining projections (K hi, Q0 hi, Q1-3, V) are split into
    PE quanta and statically scheduled into attention bands to keep the
    PE dense (HAM stays at full clock).
  - Normalize: reciprocal + per-region tensor_scalar_mul PSUM->SBUF
    staging, one batched 3D DMA out per (pair, q-block).
"""

import sys

for _p in ("/opt/trn_rl_repo",):
    if _p not in sys.path:
        sys.path.insert(0, _p)

import contextlib

import numpy as np
import ml_dtypes

import concourse.bass as bass
import concourse.tile as tile
from concourse import bacc, mybir
from concourse.bass_utils import run_bass_kernel_spmd
from concourse.masks import make_upper_triangular

BF16 = mybir.dt.bfloat16
F32 = mybir.dt.float32
I16 = mybir.dt.int16
AF = mybir.ActivationFunctionType
ALU = mybir.AluOpType

D = 2048
HS = 64
SCALE = 1.0 / 8.0  # 1/sqrt(HS)

# DVE approximate exp (Schraudolph in bf16-bit domain): the int16 value
# round(EXP_A*S + EXP_B) has the bit pattern of bf16(exp(S/8)) up to a
# +-3% ripple that softmax normalization mostly cancels.  f32->int16
# conversion on DVE rounds to nearest (HW-verified).
EXP_A = float(128.0 * 1.4426950408889634 * SCALE)
EXP_B = float(127.0 * 128.0 - 366393.0 / 65536.0 - 0.5)


def _emit_body(tc, aps, T):
    nc = tc.nc
    NT = T // 128            # k/q tiles
    QB = min(1024, T)        # q-block width
    NQB = T // QB
    QTPB = QB // 128         # q-tiles per block
    ND = D // 128            # contraction chunks
    TCW = min(512, T)        # projection t-chunk width
    NTC = T // TCW
    NREG = 2 * QTPB          # PV accum regions per (pair, q-block)

    def aoff(r):
        # 65-wide accum regions packed 7 per PSUM bank (no bank crossing)
        return 512 * (r // 7) + 65 * (r % 7)

    PVW = 512 * ((NREG + 6) // 7)

    def wmax(ki):
        return min(QB, T - 128 * ki)

    xT, wqT, wkT, wvT, cosr, sins, out = aps

    ctx = tc._kernel_exitstack = contextlib.ExitStack()

    pers = ctx.enter_context(tc.tile_pool(name="pers", bufs=1))
    rp = ctx.enter_context(tc.tile_pool(name="rope", bufs=2))
    ppts = ctx.enter_context(tc.tile_pool(name="ppts", bufs=1))
    prec = ctx.enter_context(tc.tile_pool(name="prec", bufs=2))

    # ---- tiles (3D: [128, di, cols] so one DMA covers all di chunks) ----
    xsb = pers.tile([128, ND, T], BF16, tag="xsb")
    wqsb = pers.tile([128, ND, 512], BF16, tag="wqsb")
    wksb = pers.tile([128, ND, 128], BF16, tag="wksb")
    wvsb = pers.tile([128, ND, 128], BF16, tag="wvsb")
    xTs = [xsb[:, di, :] for di in range(ND)]
    wqTs = [wqsb[:, di, :] for di in range(ND)]
    wkTs = [wksb[:, di, :] for di in range(ND)]
    wvTs = [wvsb[:, di, :] for di in range(ND)]
    cosr_t = pers.tile([128, T], BF16, tag="cosr")
    sins_t = pers.tile([128, T], BF16, tag="sins")

    # ---- priority-ordered input DMA (few big DMAs: the sync queue
    # serializes dma_start issuance at ~0.65us apiece) ----
    def load_x_cols(c0, c1):
        for dlo in range(0, ND, 8):
            nc.sync.dma_start(out=xsb[:, dlo:dlo + 8, c0:c1],
                              in_=xT[:, dlo:dlo + 8, c0:c1])

    nc.sync.dma_start(out=wksb[:], in_=wkT[:, :, :])
    for dlo in range(0, ND, 4):  # first x chunk at finer grain
        nc.sync.dma_start(out=xsb[:, dlo:dlo + 4, 0:min(512, T)],
                          in_=xT[:, dlo:dlo + 4, 0:min(512, T)])
    nc.sync.dma_start(out=wqsb[:, :, 0:128], in_=wqT[:, :, 0:128])
    nc.sync.dma_start(out=cosr_t[:], in_=cosr[:, :])
    nc.sync.dma_start(out=sins_t[:], in_=sins[:, :])
    wq_rest_loaded = False
    for c0 in range(512, T, 512):
        load_x_cols(c0, c0 + 512)
        if c0 == 1024:
            nc.sync.dma_start(out=wqsb[:, :, 128:512], in_=wqT[:, :, 128:512])
            wq_rest_loaded = True
    if not wq_rest_loaded:
        nc.sync.dma_start(out=wqsb[:, :, 128:512], in_=wqT[:, :, 128:512])
    nc.sync.dma_start(out=wvsb[:], in_=wvT[:, :, :])

    triu = pers.tile([128, 128], BF16, tag="triu")
    make_upper_triangular(nc, triu[:], val=1.0, diag=True)

    # V' tiles: [kv0 64 | one | kv1 64 | one]
    vts = []
    for ti in range(NT):
        v = pers.tile([128, 130], BF16, tag=f"v{ti}", name=f"v{ti}")
        nc.vector.memset(v[:, 64:65], 1.0)
        nc.vector.memset(v[:, 129:130], 1.0)
        vts.append(v)

    qts = [pers.tile([128, T], BF16, tag=f"qt{m}", name=f"qt{m}") for m in range(4)]
    kt = pers.tile([128, T], BF16, tag="kt")

    def rope(tgt, c0, cw, scalar_copy=False):
        """RoPE in transposed layout on tgt[:, c0:c0+cw]."""
        swp = rp.tile([128, cw], BF16, tag="swp", name="swp")
        for (a, b) in ((0, 32), (32, 0), (64, 96), (96, 64)):
            nc.gpsimd.dma_start(out=swp[a:a + 32, :],
                                in_=tgt[b:b + 32, c0:c0 + cw])
        tmp = rp.tile([128, cw], BF16, tag="tmp", name="tmp")
        nc.vector.tensor_tensor(out=tmp[:], in0=tgt[:, c0:c0 + cw],
                                in1=cosr_t[:, c0:c0 + cw], op=ALU.mult)
        nc.vector.tensor_tensor(out=swp[:], in0=swp[:],
                                in1=sins_t[:, c0:c0 + cw], op=ALU.mult)
        nc.vector.tensor_tensor(out=tgt[:, c0:c0 + cw], in0=tmp[:], in1=swp[:],
                                op=ALU.add)

    # ---- projection quanta ----
    # A "group" = one PSUM accumulation over all ND chunks; split into
    # quanta of 4 chunks.  Groups never interleave (schedule preserves
    # emission order), so the single "pj" bank has one open group.
    def kq_quanta(pool, m, tcI, scalar_copy):
        """Quanta for one t-chunk of the K (m is None) or Qm projection."""
        state = {}

        def quantum(k):
            if k == 0:
                state["ps"] = pool.tile([128, TCW], F32, tag="pj", name="pjq")
            ps = state["ps"]
            for di in range(k * 4, k * 4 + 4):
                lhs = wkTs[di][:] if m is None else \
                    wqTs[di][:, m * 128:(m + 1) * 128]
                nc.tensor.matmul(
                    ps[:], lhs, xTs[di][:, tcI * TCW:(tcI + 1) * TCW],
                    start=(di == 0), stop=(di == ND - 1))
            if k == 3:
                tgt = kt if m is None else qts[m]
                cp = nc.scalar.copy if scalar_copy else nc.vector.tensor_copy
                cp(tgt[:, tcI * TCW:(tcI + 1) * TCW], ps[:])

        return [lambda kk=k: quantum(kk) for k in range(4)]

    def v_filler(pool, ti, scalar_copy):
        def f():
            psf = pool.tile([128, TCW], F32, tag="pj", name="pjv")
            ps = psf[:, 0:128]
            for di in range(ND):
                nc.tensor.matmul(
                    ps, xTs[di][:, ti * 128:(ti + 1) * 128], wvTs[di][:],
                    start=(di == 0), stop=(di == ND - 1))
            cp = nc.scalar.copy if scalar_copy else nc.vector.tensor_copy
            cp(vts[ti][:, 0:64], ps[:, 0:64])
            cp(vts[ti][:, 65:129], ps[:, 64:128])
        return f

    # ---- phase A: first halves of K and Q0 + their rope ----
    with tc.tile_pool(name="ppe", bufs=4, space="PSUM") as ppe:
        nhalf = max(1, NTC // 2)
        for tcI in range(nhalf):
            for q in kq_quanta(ppe, None, tcI, False):
                q()
        for tcI in range(nhalf):
            for q in kq_quanta(ppe, 0, tcI, False):
                q()
        for tcI in range(nhalf):
            rope(kt, tcI * TCW, TCW)
            rope(qts[0], tcI * TCW, TCW)

    # ---- attention pools (8 PSUM banks: 2+2+3+1) ----
    sp = ctx.enter_context(tc.tile_pool(name="spsum", bufs=1, space="PSUM"))
    pvp = ctx.enter_context(tc.tile_pool(name="pvp", bufs=1, space="PSUM"))
    projp = ctx.enter_context(tc.tile_pool(name="projp", bufs=1, space="PSUM"))

    # ---- static filler schedule ----
    # sched[(m, qb)] = {"bands": {ki: [closures]}, "pre_pv": [closures]}
    sched = {(m, qb): {"bands": {}, "pre_pv": []}
             for m in range(4) for qb in range(NQB)}

    def add(m, qb, ki, f):
        sched[(m, qb)]["bands"].setdefault(ki, []).append(f)

    def band_order(m, qb):
        return list(range((qb + 1) * QTPB))

    def build_schedule():
        # stream of (m, qb, ki) slots in emission order
        slots = [(m, qb, ki)
                 for m in range(4) for qb in range(NQB)
                 for ki in band_order(m, qb)]
        bpp = NQB * QTPB + QTPB * (NQB * (NQB - 1)) // 2  # bands per pair

        # (items, deadline_band): earliest-deadline-first, placed in the
        # least-loaded band before the deadline (keeps bands ~uniform so
        # the exp pipeline is never starved or flooded).
        nhalf = max(1, NTC // 2)
        groups = []
        g = []
        for tcI in range(nhalf, NTC):  # Q0 hi cols: before qb1 of pair 0
            g.extend(kq_quanta(projp, 0, tcI, True))
            g.append(lambda tcI=tcI: rope(qts[0], tcI * TCW, TCW))
        groups.append((g, QTPB))
        g = [v_filler(projp, ti, True) for ti in range(QTPB)]
        groups.append((g, QTPB))      # V lo: before pair-0 qb0 PV
        g = []
        for tcI in range(nhalf, NTC):  # K hi cols: before qb1 band ki=QTPB
            g.extend(kq_quanta(projp, None, tcI, True))
            g.append(lambda tcI=tcI: rope(kt, tcI * TCW, TCW))
        groups.append((g, QTPB + min(QTPB, 2 * QTPB - 1)))
        g = [v_filler(projp, ti, True) for ti in range(QTPB, NT)]
        groups.append((g, bpp))       # V hi: before pair-0 qb1 PV
        for m in (1, 2, 3):           # Qm: before pair m starts
            g = []
            for tcI in range(NTC):
                g.extend(kq_quanta(projp, m, tcI, True))
                g.append(lambda m=m, tcI=tcI: rope(qts[m], tcI * TCW, TCW))
            groups.append((g, m * bpp))

        # Sequential fill (psum "pj" groups must never interleave) with
        # region capacities tuned so every group meets its deadline while
        # bands stay near-uniformly loaded.
        def cap(b):
            if b < QTPB:
                return 3
            if b < 2 * QTPB:
                return 2
            if b < bpp:
                return 3
            return 1

        bi, used = 0, 0
        for g, dl in groups:
            for it in g:
                while used >= cap(bi):
                    bi, used = bi + 1, 0
                assert bi < len(slots)
                add(*slots[bi], it)
                used += 1
            assert bi < dl, f"filler group missed deadline {dl} (at {bi})"

    build_schedule()

    # ---- paired attention ----
    def pv_closures(m, qb, ptsAs, ptsBs):
        """Per-(qt,head) PV accumulation groups + a normalize closure."""
        pv = pvp.tile([128, PVW], F32, tag="pv", name="pv")
        groups = []

        def grp(j, h, pts_d):
            def go():
                qt = qb * QTPB + j
                r = h * QTPB + j
                for ki in range(qt + 1):
                    q0 = max(128 * ki, QB * qb)
                    coloff = 128 * qt - q0
                    nc.tensor.matmul(
                        pv[:, aoff(r):aoff(r) + 65],
                        pts_d[ki][:, coloff:coloff + 128],
                        vts[ki][:, h * 65:(h + 1) * 65],
                        start=(ki == 0), stop=(ki == qt))
            return go

        for j in range(QTPB):
            groups.append(grp(j, 0, ptsAs))
            groups.append(grp(j, 1, ptsBs))

        def norm():
            rec = prec.tile([128, NREG], F32, tag="rec", name="rec")
            nb = (NREG + 6) // 7
            for b in range(nb):
                lo, hi = 7 * b, min(7 * b + 7, NREG)
                nc.vector.tensor_copy(
                    rec[:, lo:hi],
                    pv[:, 512 * b + 64:512 * b + 64 + 65 * (hi - lo):65])
            recr = prec.tile([128, NREG], F32, tag="recr", name="recr")
            nc.vector.reciprocal(recr[:], rec[:])
            stg = prec.tile([128, QTPB, 128], F32, tag="stg", name="stg")
            for j in range(QTPB):
                for h in (0, 1):
                    r = h * QTPB + j
                    nc.vector.tensor_scalar_mul(
                        stg[:, j, h * 64:h * 64 + 64],
                        pv[:, aoff(r):aoff(r) + 64],
                        recr[:, r:r + 1])
            nc.sync.dma_start(
                out=out[:, qb * QTPB:(qb + 1) * QTPB, m * 128:(m + 1) * 128],
                in_=stg[:])

        return groups, norm

    def attn_pair(m):
        pend, pend_norm = [], None
        for qb in range(NQB):
            fl = sched[(m, qb)]
            order = band_order(m, qb)
            descending = order[0] != 0

            ptsAs, ptsBs = {}, {}
            for ki in order:
                if descending and ki == QTPB - 1:
                    # low-ki bands rewrite pa{0..QTPB-1}: woven PV of the
                    # previous q-block (which reads them) must be done
                    while pend:
                        pend.pop(0)()
                    if pend_norm:
                        pend_norm()
                        pend_norm = None
                q0 = max(128 * ki, QB * qb)
                w = QB * (qb + 1) - q0
                sA = sp.tile([128, QB], F32, tag="sA", name="sA")
                sB = sp.tile([128, QB], F32, tag="sB", name="sB")
                for off in range(0, w, 512):
                    cn = min(512, w - off)
                    nc.tensor.matmul(
                        sA[:, off:off + cn],
                        kt[0:64, ki * 128:(ki + 1) * 128],
                        qts[m][0:64, q0 + off:q0 + off + cn],
                        start=True, stop=True)
                    nc.tensor.matmul(
                        sB[:, off:off + cn],
                        kt[64:128, ki * 128:(ki + 1) * 128],
                        qts[m][64:128, q0 + off:q0 + off + cn],
                        start=True, stop=True)
                ptsA = ppts.tile([128, wmax(ki)], BF16, tag=f"pa{ki}",
                                 name=f"pa{ki}")
                ptsB = ppts.tile([128, wmax(ki)], BF16, tag=f"pb{ki}",
                                 name=f"pb{ki}")
                nc.scalar.activation(ptsA[:, 0:w], sA[:, 0:w], AF.Exp,
                                     scale=SCALE)
                nc.vector.tensor_scalar(
                    out=ptsB[:, 0:w].bitcast(I16), in0=sB[:, 0:w],
                    scalar1=EXP_A, scalar2=EXP_B, op0=ALU.mult, op1=ALU.add)
                if 128 * ki >= QB * qb:  # band contains the diagonal block
                    nc.gpsimd.tensor_tensor(out=ptsA[:, 0:128],
                                            in0=ptsA[:, 0:128], in1=triu[:],
                                            op=ALU.mult)
                    nc.gpsimd.tensor_tensor(out=ptsB[:, 0:128],
                                            in0=ptsB[:, 0:128], in1=triu[:],
                                            op=ALU.mult)
                ptsAs[ki], ptsBs[ki] = ptsA, ptsB

                busy = 0
                for f in fl["bands"].get(ki, ()):
                    f()
                    busy += 1
                for _ in range(2):
                    if pend:
                        pend.pop(0)()
                        busy += 1
                if not pend and pend_norm:
                    pend_norm()
                    pend_norm = None
                if False and busy == 0 and w >= 256:
                    # PE warm-keepers: depend on this band's exp outputs
                    # (cols 128+ to avoid the gpsimd diag-mask dep) so they
                    # execute inside the exp-wait gap (HAM stays hot)
                    for pts_t in (ptsA, ptsB):
                        ps = projp.tile([128, TCW], F32, tag="pj", name="pjw")
                        nc.tensor.matmul(ps[:, 0:TCW - 16],
                                         pts_t[:, 128:256],
                                         qts[m][:, 0:TCW - 16],
                                         start=True, stop=True)

            groups, norm = pv_closures(m, qb, ptsAs, ptsBs)
            for g in groups:
                g()
            norm()
        while pend:
            pend.pop(0)()
        if pend_norm:
            pend_norm()

    for m in range(4):
        attn_pair(m)

    ctx.close()


def build_program(T=2048, num_devices=8):
    nc = bacc.Bacc("TRN2", target_bir_lowering=False, debug=False,
                   num_devices=num_devices)
    nd = D // 128
    # [p, di, c] layouts: one DMA loads a column chunk of every di tile
    xT = nc.dram_tensor("xT", (128, nd, T), BF16, kind="ExternalInput").ap()
    wqT = nc.dram_tensor("wqT", (128, nd, 512), BF16, kind="ExternalInput").ap()
    wkT = nc.dram_tensor("wkT", (128, nd, 128), BF16, kind="ExternalInput").ap()
    wvT = nc.dram_tensor("wvT", (128, nd, 128), BF16, kind="ExternalInput").ap()
    cosr = nc.dram_tensor("cosr", (128, T), BF16, kind="ExternalInput").ap()
    sins = nc.dram_tensor("sins", (128, T), BF16, kind="ExternalInput").ap()
    # out[p, qt, c]: row qt*128+p of the logical [T, 512] output; columns
    # pair-major: pair m, head h (0=m, 1=m+4) at c in [128m+64h, +64)
    out = nc.dram_tensor("out", (128, T // 128, 512), F32,
                         kind="ExternalOutput").ap()
    with tile.TileContext(nc) as tc:
        _emit_body(tc, (xT, wqT, wkT, wvT, cosr, sins, out), T)
    nc.compile()
    return nc


# ---------------- host side ----------------

def _qperm(j):
    rows = []
    for m in range(4):
        for r in range(128):
            h = m if r < 64 else m + 4
            d = 2 * (r % 32) + (1 if (r % 64) >= 32 else 0)
            rows.append((8 * j + h) * 64 + d)
    return np.array(rows)


def _kperm(j):
    rows = []
    for kv in range(2):
        for r in range(64):
            d = 2 * (r % 32) + (1 if r >= 32 else 0)
            rows.append((2 * j + kv) * 64 + d)
    return np.array(rows)


def _to3d(a):
    """[D, C] -> [128, D//128, C] (partition-major di stacking)."""
    Dd, C = a.shape
    return np.ascontiguousarray(a.reshape(Dd // 128, 128, C).transpose(1, 0, 2))


def make_core_inputs(x, Wq, Wk, Wv, cos, sin):
    """Per-core input dicts (host prep). x: [B,T,D]."""
    bf = ml_dtypes.bfloat16
    B, T, _ = x.shape
    xTb = [_to3d(np.ascontiguousarray(x[b].T).astype(bf)) for b in range(B)]
    cosT = np.ascontiguousarray(cos.T.astype(np.float32))  # [32, T]
    sinT = np.ascontiguousarray(sin.T.astype(np.float32))
    cosr = np.tile(cosT, (4, 1)).astype(bf)
    sgn = np.repeat(np.array([-1.0, 1.0, -1.0, 1.0], np.float32), 32)
    sins = (np.tile(sinT, (4, 1)) * sgn[:, None]).astype(bf)
    maps = []
    for c in range(8):
        b, j = c // 4, c % 4
        maps.append({
            "xT": xTb[b],
            "wqT": _to3d(Wq[_qperm(j)].T.astype(bf)),
            "wkT": _to3d(Wk[_kperm(j)].T.astype(bf)),
            "wvT": _to3d(Wv[128 * j:128 * (j + 1)].T.astype(bf)),
            "cosr": cosr,
            "sins": sins,
        })
    return maps


def core_out_to_full(res_out):
    """res_out: [128, NT, 512] pair-major -> [T, 512] head-major."""
    nt = res_out.shape[1]
    o = np.transpose(res_out, (1, 0, 2)).reshape(nt * 128, 512)
    full = np.empty_like(o)
    for m in range(4):
        for h in range(2):
            full[:, (m + 4 * h) * 64:(m + 4 * h) * 64 + 64] = \
                o[:, (m * 2 + h) * 64:(m * 2 + h) * 64 + 64]
    return full


_CACHE = {}


def _get_program():
    if "nc" not in _CACHE:
        _CACHE["nc"] = build_program(T=2048, num_devices=8)
    return _CACHE["nc"]


def run_on_hw(in_maps, trace=False):
    nc = _get_program()
    return run_bass_kernel_spmd(nc, in_maps, list(range(8)), trace=trace)


def kernel(x, Wq, Wk, Wv, cos, sin):
    x = np.asarray(x, np.float32)
    Wq = np.asarray(Wq, np.float32)
    Wk = np.asarray(Wk, np.float32)
    Wv = np.asarray(Wv, np.float32)
    cos = np.asarray(cos, np.float32)
    sin = np.asarray(sin, np.float32)
    maps = make_core_inputs(x, Wq, Wk, Wv, cos, sin)
    res = run_on_hw(maps, trace=False)
    B, T = x.shape[0], x.shape[1]
    out = np.empty((B, T, 2048), np.float32)
    for c in range(8):
        b, j = c // 4, c % 4
        out[b, :, 512 * j:512 * (j + 1)] = core_out_to_full(res.results[c]["out"])
    return out



# revision 2
# speedup vs baseline: 1.1418x; 1.1418x over previous
"""GQA kernel v3 for Trainium2 (Bass/Tile), 8 NeuronCores.

Sharding: core c -> batch b=c//4, kv-head pair j=c%4 (kv heads 2j,2j+1,
q heads 8j..8j+7).  Each core computes out[b, :, 512j:512(j+1)] (pair-major
column order + per-head softmax denominator; divided + untangled on host).

v3 changes over the 288us baseline (trace-driven):
  - Input DMA split across the two HWDGE queues (sync + scalar) instead
    of 48us serialized on sync; early-needed tensors go on scalar.
  - PE warm-up matmuls at t=0 (triu x triu) so the HAM clock-gate is at
    8/8 before the first projection matmul.
  - S PSUM ring (one tag, bufs=3, 512-wide chunks) decouples S matmuls
    from exp consumption (baseline re-used one buffer per band, chaining
    S(k+1) on exp(k) and causing ~1.2us PE stalls per band -> HAM
    re-throttle to half clock for the last ~100us).
  - PV accumulation groups are woven into the band loop (group qt right
    after band qt+2) instead of an end-of-block burst: uniform PE load,
    exp engines never starve, short kernel tail.
  - RoPE partition swap via a signed-permutation matmul on the PE
    (host-supplied P) instead of 57us of gpsimd SBUF-SBUF DMAs.
  - Softmax division moved to the host (numerators + ones-column
    denominator are DMA'd out raw): removes DVE reciprocal/normalize.
  - exp split per band as before: head m via ACT (exact), head m+4 via
    DVE int16 Schraudolph writing bf16 bit patterns.
"""

import sys

for _p in ("/opt/trn_rl_repo",):
    if _p not in sys.path:
        sys.path.insert(0, _p)

import contextlib

import numpy as np
import ml_dtypes

import concourse.bass as bass
import concourse.tile as tile
from concourse import bacc, mybir
from concourse.bass_utils import run_bass_kernel_spmd
from concourse.masks import make_upper_triangular

BF16 = mybir.dt.bfloat16
F32 = mybir.dt.float32
I16 = mybir.dt.int16
AF = mybir.ActivationFunctionType
ALU = mybir.AluOpType

D = 2048
HS = 64
SCALE = 1.0 / 8.0  # 1/sqrt(HS)

# DVE approximate exp (Schraudolph in bf16-bit domain): the int16 value
# round(EXP_A*S + EXP_B) has the bit pattern of bf16(exp(S/8)) up to a
# +-3% ripple that softmax normalization mostly cancels.
EXP_A = float(128.0 * 1.4426950408889634 * SCALE)
EXP_B = float(127.0 * 128.0 - 366393.0 / 65536.0 - 0.5)

WARMUP_MMS = 48
LAG = 2


def _emit_body(tc, aps, T):
    nc = tc.nc
    NT = T // 128            # k/q tiles
    QB = min(1024, T)        # q-block width
    NQB = T // QB
    QTPB = QB // 128         # q-tiles per block
    ND = D // 128            # contraction chunks
    TCW = min(512, T)        # projection t-chunk width
    NTC = T // TCW
    NREG = 2 * QTPB          # PV accum regions per block
    BPP = sum((qb + 1) * QTPB for qb in range(NQB))  # bands per pair
    NBANDS = 4 * BPP

    def aoff(r):
        # 65-wide accum regions packed 7 per PSUM bank (no bank crossing)
        return 512 * (r // 7) + 65 * (r % 7)

    PVW = 512 * ((NREG + 6) // 7)

    def wmax(ki):
        return min(QB, T - 128 * ki)

    xT, wqT, wkT, wvT, cosr, sins, permM, out = aps

    ctx = tc._kernel_exitstack = contextlib.ExitStack()

    pers = ctx.enter_context(tc.tile_pool(name="pers", bufs=1))
    rp = ctx.enter_context(tc.tile_pool(name="rope", bufs=2))
    ppts = ctx.enter_context(tc.tile_pool(name="ppts", bufs=1))
    stgp = ctx.enter_context(tc.tile_pool(name="stgp", bufs=2))

    # ---- persistent tiles ----
    xsb = pers.tile([128, ND, T], BF16, tag="xsb")
    wqsb = pers.tile([128, ND, 512], BF16, tag="wqsb")
    wksb = pers.tile([128, ND, 128], BF16, tag="wksb")
    wvsb = pers.tile([128, ND, 128], BF16, tag="wvsb")
    xTs = [xsb[:, di, :] for di in range(ND)]
    wqTs = [wqsb[:, di, :] for di in range(ND)]
    wkTs = [wksb[:, di, :] for di in range(ND)]
    wvTs = [wvsb[:, di, :] for di in range(ND)]
    cosr_t = pers.tile([128, T], BF16, tag="cosr")
    sins_t = pers.tile([128, T], BF16, tag="sins")
    perm_t = pers.tile([128, 128], BF16, tag="perm")

    # ---- input DMA: scalar (ACT HWDGE) carries the early tensors + the
    # di-hi half of the first x chunks; sync carries the bulk. ----
    hnd = ND // 2
    nc.scalar.dma_start(out=perm_t[:], in_=permM[:, :])
    nc.scalar.dma_start(out=wksb[:], in_=wkT[:, :, :])
    nc.sync.dma_start(out=xsb[:, 0:hnd, 0:TCW], in_=xT[:, 0:hnd, 0:TCW])
    nc.scalar.dma_start(out=xsb[:, hnd:ND, 0:TCW], in_=xT[:, hnd:ND, 0:TCW])
    nc.scalar.dma_start(out=wqsb[:, :, 0:128], in_=wqT[:, :, 0:128])
    nc.scalar.dma_start(out=cosr_t[:], in_=cosr[:, :])
    nc.scalar.dma_start(out=sins_t[:], in_=sins[:, :])
    for c0 in range(TCW, T, TCW):
        nc.sync.dma_start(out=xsb[:, 0:hnd, c0:c0 + TCW],
                          in_=xT[:, 0:hnd, c0:c0 + TCW])
        nc.scalar.dma_start(out=xsb[:, hnd:ND, c0:c0 + TCW],
                            in_=xT[:, hnd:ND, c0:c0 + TCW])
        if c0 == TCW:
            nc.sync.dma_start(out=wvsb[:], in_=wvT[:, :, :])
    nc.sync.dma_start(out=wqsb[:, :, 128:512], in_=wqT[:, :, 128:512])

    triu = pers.tile([128, 128], BF16, tag="triu")
    make_upper_triangular(nc, triu[:], val=1.0, diag=True)

    # V' tiles: [kv0 64 | one | kv1 64 | one]
    vts = []
    for ti in range(NT):
        v = pers.tile([128, 130], BF16, tag=f"v{ti}", name=f"v{ti}")
        nc.vector.memset(v[:, 64:65], 1.0)
        nc.vector.memset(v[:, 129:130], 1.0)
        vts.append(v)

    qts = [pers.tile([128, T], BF16, tag=f"qt{m}", name=f"qt{m}") for m in range(4)]
    kt = pers.tile([128, T], BF16, tag="kt")

    def rope(pool, tgt, c0, cw):
        """RoPE on tgt[:, c0:c0+cw]: partition swap via perm matmul on PE,
        then 3 DVE passes (sign is folded into sins host-side)."""
        pp = pool.tile([128, cw], F32, tag="pj", name="ropeps")
        nc.tensor.matmul(pp[:], perm_t[:], tgt[:, c0:c0 + cw],
                         start=True, stop=True)
        swp = rp.tile([128, cw], BF16, tag="swp", name="swp")
        nc.vector.tensor_tensor(out=swp[:], in0=pp[:],
                                in1=sins_t[:, c0:c0 + cw], op=ALU.mult)
        tmp = rp.tile([128, cw], BF16, tag="tmp", name="tmp")
        nc.vector.tensor_tensor(out=tmp[:], in0=tgt[:, c0:c0 + cw],
                                in1=cosr_t[:, c0:c0 + cw], op=ALU.mult)
        nc.vector.tensor_tensor(out=tgt[:, c0:c0 + cw], in0=tmp[:], in1=swp[:],
                                op=ALU.add)

    def kq_quanta(pool, m, tcI, copy_eng):
        """4 quanta for one t-chunk of the K (m is None) or Qm projection."""
        state = {}

        def quantum(k):
            if k == 0:
                state["ps"] = pool.tile([128, TCW], F32, tag="pj", name="pjq")
            ps = state["ps"]
            for di in range(k * 4, k * 4 + 4):
                lhs = wkTs[di][:] if m is None else \
                    wqTs[di][:, m * 128:(m + 1) * 128]
                nc.tensor.matmul(
                    ps[:], lhs, xTs[di][:, tcI * TCW:(tcI + 1) * TCW],
                    start=(di == 0), stop=(di == ND - 1))
            if k == 3:
                tgt = kt if m is None else qts[m]
                copy_eng(tgt[:, tcI * TCW:(tcI + 1) * TCW], ps[:])

        return [lambda kk=k: quantum(kk) for k in range(4)]

    def v_filler(pool, ti, copy_eng):
        def f():
            psf = pool.tile([128, TCW], F32, tag="pj", name="pjv")
            ps = psf[:, 0:128]
            for di in range(ND):
                nc.tensor.matmul(
                    ps, xTs[di][:, ti * 128:(ti + 1) * 128], wvTs[di][:],
                    start=(di == 0), stop=(di == ND - 1))
            copy_eng(vts[ti][:, 0:64], ps[:, 0:64])
            copy_eng(vts[ti][:, 65:129], ps[:, 64:128])
        return f

    # ---- phase A: PE warm-up, then K lo + all of Q0 + V0-3 (+ropes) ----
    nhalf = max(1, NTC // 2)
    with tc.tile_pool(name="ppe", bufs=4, space="PSUM") as ppe:
        warm = ppe.tile([128, 128], F32, tag="warm", bufs=1, name="warm")
        for _ in range(WARMUP_MMS):
            nc.tensor.matmul(warm[:], triu[:], triu[:], start=True, stop=True)

        vcopy = nc.vector.tensor_copy
        nq = 4  # V tiles done in phase A
        seq = []
        seq.append(("k", 0))
        seq.append(("q", 0))
        for tcI in range(1, nhalf):
            seq.append(("k", tcI))
            seq.append(("q", tcI))
        for tcI in range(nhalf, NTC):
            if tcI == NTC - 1:
                for ti in range(min(nq, NT)):
                    seq.append(("v", ti))
            seq.append(("q", tcI))
        for kind, i in seq:
            if kind == "k":
                for q in kq_quanta(ppe, None, i, vcopy):
                    q()
                rope(ppe, kt, i * TCW, TCW)
            elif kind == "q":
                for q in kq_quanta(ppe, 0, i, vcopy):
                    q()
                rope(ppe, qts[0], i * TCW, TCW)
            else:
                v_filler(ppe, i, vcopy)()

    # ---- attention pools (8 PSUM banks: 3 + 3 + 2) ----
    sp = ctx.enter_context(tc.tile_pool(name="spsum", bufs=3, space="PSUM"))
    pvp = ctx.enter_context(tc.tile_pool(name="pvp", bufs=1, space="PSUM"))
    projp = ctx.enter_context(tc.tile_pool(name="projp", bufs=2, space="PSUM"))

    # ---- filler schedule: EDF into global bands with per-band budgets ----
    def band_of(g):
        m, r = divmod(g, BPP)
        for qb in range(NQB):
            n = (qb + 1) * QTPB
            if r < n:
                return m, qb, r
            r -= n
        raise AssertionError

    def committed(g):
        _, qb, ki = band_of(g)
        q0 = max(128 * ki, QB * qb)
        w = QB * (qb + 1) - q0
        c = w / 2400.0
        wqt = ki - LAG
        if wqt >= qb * QTPB:
            c += (wqt + 1) * 2 * 0.035
        return c

    def cap(g):
        return 2.6 if g < BPP else 1.9

    scopy = nc.scalar.copy
    groups = []  # (deadline_band, [(cost_us, closure)])
    for tcI in range(nhalf, NTC):
        items = [(0.9, q) for q in kq_quanta(projp, None, tcI, scopy)]
        items.append((0.3, (lambda tcI=tcI: rope(projp, kt, tcI * TCW, TCW))))
        groups.append((QTPB + tcI * (TCW // 128), items))
    for ti in range(min(4, NT), NT):
        qb = ti // QTPB
        qoff = sum((q + 1) * QTPB for q in range(qb))
        dl = qoff + min(ti - qb * QTPB + qb * QTPB + LAG, (qb + 1) * QTPB - 1)
        groups.append((dl, [(0.95, v_filler(projp, ti, scopy))]))
    for m in (1, 2, 3):
        for tcI in range(NTC):
            items = [(0.9, q) for q in kq_quanta(projp, m, tcI, scopy)]
            items.append(
                (0.3, (lambda m=m, tcI=tcI: rope(projp, qts[m], tcI * TCW, TCW))))
            dl = m * BPP + (0 if tcI * TCW < QB else QTPB)
            groups.append((dl, items))

    sched = {}
    groups.sort(key=lambda x: x[0])
    g, used = 0, 0.0
    for dl, items in groups:
        for cost, fn in items:
            budget = max(0.4, cap(g) - committed(g))
            if used >= budget:
                g, used = g + 1, 0.0
                assert g < NBANDS, "filler overflow"
            assert g < dl, f"filler deadline violated: band {g} >= {dl}"
            sched.setdefault(g, []).append(fn)
            used += cost

    # ---- attention blocks with woven PV ----
    def block(m, qb):
        top = (qb + 1) * QTPB - 1
        qoff = sum((q + 1) * QTPB for q in range(qb))
        state = {"pv": None}
        ptsAs, ptsBs = {}, {}

        def emit_group(qt):
            if state["pv"] is None:
                state["pv"] = pvp.tile([128, PVW], F32, tag="pv", name="pv")
            pv = state["pv"]
            for h, pts_d in ((0, ptsAs), (1, ptsBs)):
                r = h * QTPB + (qt - qb * QTPB)
                for ki in range(qt + 1):
                    q0 = max(128 * ki, QB * qb)
                    coloff = 128 * qt - q0
                    nc.tensor.matmul(
                        pv[:, aoff(r):aoff(r) + 65],
                        pts_d[ki][:, coloff:coloff + 128],
                        vts[ki][:, h * 65:(h + 1) * 65],
                        start=(ki == 0), stop=(ki == qt))

        for ki in range(top + 1):
            q0 = max(128 * ki, QB * qb)
            w = QB * (qb + 1) - q0
            ptsA = ppts.tile([128, wmax(ki)], BF16, tag=f"pa{ki}",
                             name=f"pa{ki}")
            ptsB = ppts.tile([128, wmax(ki)], BF16, tag=f"pb{ki}",
                             name=f"pb{ki}")
            for off in range(0, w, TCW):
                cn = min(TCW, w - off)
                sA = sp.tile([128, TCW], F32, tag="s", name="sA")
                sB = sp.tile([128, TCW], F32, tag="s", name="sB")
                nc.tensor.matmul(
                    sA[:, 0:cn],
                    kt[0:64, ki * 128:(ki + 1) * 128],
                    qts[m][0:64, q0 + off:q0 + off + cn],
                    start=True, stop=True)
                nc.tensor.matmul(
                    sB[:, 0:cn],
                    kt[64:128, ki * 128:(ki + 1) * 128],
                    qts[m][64:128, q0 + off:q0 + off + cn],
                    start=True, stop=True)
                nc.scalar.activation(ptsA[:, off:off + cn], sA[:, 0:cn],
                                     AF.Exp, scale=SCALE)
                nc.vector.tensor_scalar(
                    out=ptsB[:, off:off + cn].bitcast(I16), in0=sB[:, 0:cn],
                    scalar1=EXP_A, scalar2=EXP_B, op0=ALU.mult, op1=ALU.add)
            if 128 * ki >= QB * qb:  # band contains the diagonal block
                nc.gpsimd.tensor_tensor(out=ptsA[:, 0:128],
                                        in0=ptsA[:, 0:128], in1=triu[:],
                                        op=ALU.mult)
                nc.gpsimd.tensor_tensor(out=ptsB[:, 0:128],
                                        in0=ptsB[:, 0:128], in1=triu[:],
                                        op=ALU.mult)
            ptsAs[ki], ptsBs[ki] = ptsA, ptsB

            for f in sched.get(m * BPP + qoff + ki, ()):
                f()

            wqt = ki - LAG
            if wqt >= qb * QTPB:
                emit_group(wqt)

        for qt in range(max(qb * QTPB, top + 1 - LAG), top + 1):
            emit_group(qt)

        # raw numerators + denominators to SBUF, one batched DMA out
        stg = stgp.tile([128, QTPB, 130], F32, tag="stg", name="stg")
        pv = state["pv"]
        for j in range(QTPB):
            for h in (0, 1):
                r = h * QTPB + j
                eng = nc.scalar.copy if h == 0 else nc.vector.tensor_copy
                eng(stg[:, j, h * 65:(h + 1) * 65],
                    pv[:, aoff(r):aoff(r) + 65])
        nc.sync.dma_start(
            out=out[:, qb * QTPB:(qb + 1) * QTPB, m * 130:(m + 1) * 130],
            in_=stg[:])

    for m in range(4):
        for qb in range(NQB):
            block(m, qb)

    ctx.close()


def build_program(T=2048, num_devices=8):
    nc = bacc.Bacc("TRN2", target_bir_lowering=False, debug=False,
                   num_devices=num_devices)
    nd = D // 128
    xT = nc.dram_tensor("xT", (128, nd, T), BF16, kind="ExternalInput").ap()
    wqT = nc.dram_tensor("wqT", (128, nd, 512), BF16, kind="ExternalInput").ap()
    wkT = nc.dram_tensor("wkT", (128, nd, 128), BF16, kind="ExternalInput").ap()
    wvT = nc.dram_tensor("wvT", (128, nd, 128), BF16, kind="ExternalInput").ap()
    cosr = nc.dram_tensor("cosr", (128, T), BF16, kind="ExternalInput").ap()
    sins = nc.dram_tensor("sins", (128, T), BF16, kind="ExternalInput").ap()
    permM = nc.dram_tensor("perm", (128, 128), BF16, kind="ExternalInput").ap()
    # out[p, qt, c]: row qt*128+p of the logical [T, 520] output; columns
    # pair-major: pair m, head h (0=m, 1=m+4) numerator at
    # c in [130m+65h, +64), denominator at 130m+65h+64
    out = nc.dram_tensor("out", (128, T // 128, 520), F32,
                         kind="ExternalOutput").ap()
    with tile.TileContext(nc) as tc:
        _emit_body(tc, (xT, wqT, wkT, wvT, cosr, sins, permM, out), T)
    nc.compile()
    return nc


# ---------------- host side ----------------

def _qperm(j):
    rows = []
    for m in range(4):
        for r in range(128):
            h = m if r < 64 else m + 4
            d = 2 * (r % 32) + (1 if (r % 64) >= 32 else 0)
            rows.append((8 * j + h) * 64 + d)
    return np.array(rows)


def _kperm(j):
    rows = []
    for kv in range(2):
        for r in range(64):
            d = 2 * (r % 32) + (1 if r >= 32 else 0)
            rows.append((2 * j + kv) * 64 + d)
    return np.array(rows)


def _to3d(a):
    """[D, C] -> [128, D//128, C] (partition-major di stacking)."""
    Dd, C = a.shape
    return np.ascontiguousarray(a.reshape(Dd // 128, 128, C).transpose(1, 0, 2))


def _perm_mat():
    p = np.zeros((128, 128), dtype=ml_dtypes.bfloat16)
    for i in range(128):
        j = i + 32 if (i % 64) < 32 else i - 32
        p[i, j] = 1.0
    return p


def make_core_inputs(x, Wq, Wk, Wv, cos, sin):
    """Per-core input dicts (host prep). x: [B,T,D]."""
    bf = ml_dtypes.bfloat16
    B, T, _ = x.shape
    xTb = [_to3d(np.ascontiguousarray(x[b].T).astype(bf)) for b in range(B)]
    cosT = np.ascontiguousarray(cos.T.astype(np.float32))  # [32, T]
    sinT = np.ascontiguousarray(sin.T.astype(np.float32))
    cosr = np.tile(cosT, (4, 1)).astype(bf)
    sgn = np.repeat(np.array([-1.0, 1.0, -1.0, 1.0], np.float32), 32)
    sins = (np.tile(sinT, (4, 1)) * sgn[:, None]).astype(bf)
    perm = _perm_mat()
    maps = []
    for c in range(8):
        b, j = c // 4, c % 4
        maps.append({
            "xT": xTb[b],
            "wqT": _to3d(Wq[_qperm(j)].T.astype(bf)),
            "wkT": _to3d(Wk[_kperm(j)].T.astype(bf)),
            "wvT": _to3d(Wv[128 * j:128 * (j + 1)].T.astype(bf)),
            "cosr": cosr,
            "sins": sins,
            "perm": perm,
        })
    return maps


def core_out_to_full(res_out):
    """res_out: [128, NT, 520] pair-major num/den -> [T, 512] head-major."""
    nt = res_out.shape[1]
    o = np.transpose(res_out, (1, 0, 2)).reshape(nt * 128, 520)
    full = np.empty((nt * 128, 512), np.float32)
    for m in range(4):
        for h in (0, 1):
            base = m * 130 + h * 65
            num = o[:, base:base + 64]
            den = o[:, base + 64:base + 65]
            full[:, (m + 4 * h) * 64:(m + 4 * h) * 64 + 64] = num / den
    return full


_CACHE = {}


def _get_program():
    if "nc" not in _CACHE:
        _CACHE["nc"] = build_program(T=2048, num_devices=8)
    return _CACHE["nc"]


def run_on_hw(in_maps, trace=False):
    nc = _get_program()
    return run_bass_kernel_spmd(nc, in_maps, list(range(8)), trace=trace)


def kernel(x, Wq, Wk, Wv, cos, sin):
    x = np.asarray(x, np.float32)
    Wq = np.asarray(Wq, np.float32)
    Wk = np.asarray(Wk, np.float32)
    Wv = np.asarray(Wv, np.float32)
    cos = np.asarray(cos, np.float32)
    sin = np.asarray(sin, np.float32)
    maps = make_core_inputs(x, Wq, Wk, Wv, cos, sin)
    res = run_on_hw(maps, trace=False)
    B, T = x.shape[0], x.shape[1]
    out = np.empty((B, T, 2048), np.float32)
    for c in range(8):
        b, j = c // 4, c % 4
        out[b, :, 512 * j:512 * (j + 1)] = core_out_to_full(res.results[c]["out"])
    return out


# revision 4
# speedup vs baseline: 1.3307x; 1.1654x over previous
"""GQA kernel v4 for Trainium2 (Bass/Tile), 8 NeuronCores.

Sharding: core c -> batch b=c//4, kv-head pair j=c%4 (kv heads 2j,2j+1,
q heads 8j..8j+7).  Each core computes out[b, :, 512j:512(j+1)] (pair-major
column order + per-head softmax denominator; divided + untangled on host).

v4 structure (trace-driven, from the 288us baseline):
  - ONE attention block per head-pair (q-block = full T): bands ki=0..15,
    each band is S^T(ki) over q cols [128ki, T).  PV accumulation group
    for q-tile qt is woven right after band qt+LAG, so PV matmuls spread
    across the whole pair instead of bursting at block end (the v3
    qb-split left 10-band "deserts" with no PE work -> HAM half-clock).
  - PV regions live in a ring of [128,512] PSUM bank tiles (7 x 65-wide
    regions each, bufs=2); each group's [128,65] numerator+denominator
    is copied to SBUF by GPSIMD right after the group closes (ACT/DVE
    stay free for exp; no end-of-block copy chain).
  - S PSUM ring (tag "s", bufs=4, 512-wide chunks) decouples S matmuls
    from exp consumption.
  - Input DMA split across the two HWDGE queues (sync + scalar),
    priority-ordered so phase A streams just-in-time.
  - PE warm-up matmuls at t=0 (triu x triu) warm the HAM clock gate.
  - RoPE partition swap via a signed-permutation matmul on the PE.
  - Softmax division on the host (numerators + ones-column denominator).
  - exp split per band: head m via ACT (exact), head m+4 via DVE int16
    Schraudolph writing bf16 bit patterns.
"""

import sys

for _p in ("/opt/trn_rl_repo",):
    if _p not in sys.path:
        sys.path.insert(0, _p)

import contextlib

import numpy as np
import ml_dtypes

import concourse.bass as bass
import concourse.tile as tile
from concourse import bacc, mybir
from concourse.bass_utils import run_bass_kernel_spmd
from concourse.masks import make_upper_triangular

BF16 = mybir.dt.bfloat16
F32 = mybir.dt.float32
I16 = mybir.dt.int16
AF = mybir.ActivationFunctionType
ALU = mybir.AluOpType

D = 2048
HS = 64
SCALE = 1.0 / 8.0  # 1/sqrt(HS)

EXP_A = float(128.0 * 1.4426950408889634 * SCALE)
EXP_B = float(127.0 * 128.0 - 366393.0 / 65536.0 - 0.5)

WARMUP_MMS = 44
LAG = 2


def _emit_body(tc, aps, T):
    nc = tc.nc
    NT = T // 128            # k/q tiles == bands per pair
    ND = D // 128            # contraction chunks
    TCW = min(512, T)        # projection t-chunk width
    NTC = T // TCW
    SCW = min(512, T)        # S-chunk width
    NBANDS = 4 * NT

    xT, wqT, wkT, wvT, cosr, sins, permM, out = aps

    ctx = tc._kernel_exitstack = contextlib.ExitStack()

    pers = ctx.enter_context(tc.tile_pool(name="pers", bufs=1))
    rp = ctx.enter_context(tc.tile_pool(name="rope", bufs=2))
    ppts = ctx.enter_context(tc.tile_pool(name="ppts", bufs=1))
    stgp = ctx.enter_context(tc.tile_pool(name="stgp", bufs=1))
    qpool = ctx.enter_context(tc.tile_pool(name="qpool", bufs=2))

    # ---- persistent tiles ----
    xsb = pers.tile([128, ND, T], BF16, tag="xsb")
    wqsb = pers.tile([128, ND, 512], BF16, tag="wqsb")
    wksb = pers.tile([128, ND, 128], BF16, tag="wksb")
    wvsb = pers.tile([128, ND, 128], BF16, tag="wvsb")
    xTs = [xsb[:, di, :] for di in range(ND)]
    wqTs = [wqsb[:, di, :] for di in range(ND)]
    wkTs = [wksb[:, di, :] for di in range(ND)]
    wvTs = [wvsb[:, di, :] for di in range(ND)]
    cosr_t = pers.tile([128, T], BF16, tag="cosr")
    sins_t = pers.tile([128, T], BF16, tag="sins")
    perm_t = pers.tile([128, 128], BF16, tag="perm")

    # ---- input DMA: scalar (ACT HWDGE) carries early tensors + di-hi x
    # halves; sync carries di-lo x halves + the late bulk. ----
    hnd = ND // 2
    nc.scalar.dma_start(out=perm_t[:], in_=permM[:, :])
    nc.scalar.dma_start(out=wksb[:], in_=wkT[:, :, :])
    nc.sync.dma_start(out=xsb[:, 0:hnd, 0:TCW], in_=xT[:, 0:hnd, 0:TCW])
    nc.scalar.dma_start(out=xsb[:, hnd:ND, 0:TCW], in_=xT[:, hnd:ND, 0:TCW])
    nc.scalar.dma_start(out=wqsb[:, :, 0:128], in_=wqT[:, :, 0:128])
    nc.scalar.dma_start(out=cosr_t[:], in_=cosr[:, :])
    nc.scalar.dma_start(out=sins_t[:], in_=sins[:, :])
    for c0 in range(TCW, T, TCW):
        nc.sync.dma_start(out=xsb[:, 0:hnd, c0:c0 + TCW],
                          in_=xT[:, 0:hnd, c0:c0 + TCW])
        nc.scalar.dma_start(out=xsb[:, hnd:ND, c0:c0 + TCW],
                            in_=xT[:, hnd:ND, c0:c0 + TCW])
        if c0 == TCW:
            nc.sync.dma_start(out=wvsb[:], in_=wvT[:, :, :])
    nc.sync.dma_start(out=wqsb[:, :, 128:512], in_=wqT[:, :, 128:512])

    triu = pers.tile([128, 128], BF16, tag="triu")
    make_upper_triangular(nc, triu[:], val=1.0, diag=True)

    # V' tiles: [kv0 64 | one | kv1 64 | one]
    vts = []
    for ti in range(NT):
        v = pers.tile([128, 130], BF16, tag=f"v{ti}", name=f"v{ti}")
        nc.vector.memset(v[:, 64:65], 1.0)
        nc.vector.memset(v[:, 129:130], 1.0)
        vts.append(v)

    kt = pers.tile([128, T], BF16, tag="kt")
    qtile = {}

    def get_qt(m):
        if m not in qtile:
            qtile[m] = qpool.tile([128, T], BF16, tag="qt", name=f"qt{m}")
        return qtile[m]

    def rope(pool, tgt, c0, cw):
        """RoPE on tgt[:, c0:c0+cw]: partition swap via perm matmul on PE,
        then 3 DVE passes (sign is folded into sins host-side)."""
        pp = pool.tile([128, cw], F32, tag="pj", name="ropeps")
        nc.tensor.matmul(pp[:], perm_t[:], tgt[:, c0:c0 + cw],
                         start=True, stop=True)
        swp = rp.tile([128, cw], BF16, tag="swp", name="swp")
        nc.vector.tensor_tensor(out=swp[:], in0=pp[:],
                                in1=sins_t[:, c0:c0 + cw], op=ALU.mult)
        tmp = rp.tile([128, cw], BF16, tag="tmp", name="tmp")
        nc.vector.tensor_tensor(out=tmp[:], in0=tgt[:, c0:c0 + cw],
                                in1=cosr_t[:, c0:c0 + cw], op=ALU.mult)
        nc.vector.tensor_tensor(out=tgt[:, c0:c0 + cw], in0=tmp[:], in1=swp[:],
                                op=ALU.add)

    def kq_quanta(pool, m, tcI, copy_eng):
        """4 quanta for one t-chunk of the K (m is None) or Qm projection."""
        state = {}

        def quantum(k):
            if k == 0:
                state["ps"] = pool.tile([128, TCW], F32, tag="pj", name="pjq")
            ps = state["ps"]
            for di in range(k * 4, k * 4 + 4):
                lhs = wkTs[di][:] if m is None else \
                    wqTs[di][:, m * 128:(m + 1) * 128]
                nc.tensor.matmul(
                    ps[:], lhs, xTs[di][:, tcI * TCW:(tcI + 1) * TCW],
                    start=(di == 0), stop=(di == ND - 1))
            if k == 3:
                tgt = kt if m is None else get_qt(m)
                copy_eng(tgt[:, tcI * TCW:(tcI + 1) * TCW], ps[:])

        return [lambda kk=k: quantum(kk) for k in range(4)]

    def v_filler(pool, ti, copy_eng):
        def f():
            psf = pool.tile([128, TCW], F32, tag="pj", name="pjv")
            ps = psf[:, 0:128]
            for di in range(ND):
                nc.tensor.matmul(
                    ps, xTs[di][:, ti * 128:(ti + 1) * 128], wvTs[di][:],
                    start=(di == 0), stop=(di == ND - 1))
            copy_eng(vts[ti][:, 0:64], ps[:, 0:64])
            copy_eng(vts[ti][:, 65:129], ps[:, 64:128])
        return f

    # ---- phase A: PE warm-up, K c0, all of Q0, V0-3 (+ropes), JIT with
    # the x chunk arrival order ----
    with tc.tile_pool(name="ppe", bufs=4, space="PSUM") as ppe:
        warm = ppe.tile([128, 128], F32, tag="warm", bufs=1, name="warm")
        for _ in range(WARMUP_MMS):
            nc.tensor.matmul(warm[:], triu[:], triu[:], start=True, stop=True)

        vcopy = nc.vector.tensor_copy
        seq = [("k", 0), ("q", 0)]
        for tcI in range(1, NTC):
            if tcI == 2:
                seq += [("v", 0), ("v", 1)]
            if tcI == NTC - 1:
                seq += [("v", 2), ("v", 3)]
            seq.append(("q", tcI))
        for kind, i in seq:
            if kind == "k":
                for q in kq_quanta(ppe, None, i, vcopy):
                    q()
                rope(ppe, kt, i * TCW, TCW)
            elif kind == "q":
                for q in kq_quanta(ppe, 0, i, vcopy):
                    q()
                rope(ppe, get_qt(0), i * TCW, TCW)
            else:
                v_filler(ppe, i, vcopy)()

    # ---- attention pools (8 PSUM banks: 4 + 2 + 2) ----
    sp = ctx.enter_context(tc.tile_pool(name="spsum", bufs=4, space="PSUM"))
    pvp = ctx.enter_context(tc.tile_pool(name="pvp", bufs=2, space="PSUM"))
    projp = ctx.enter_context(tc.tile_pool(name="projp", bufs=2, space="PSUM"))

    # ---- filler schedule: EDF into global bands with per-band budgets ----
    def committed(g):
        ki = g % NT
        c = (T - 128 * ki) / 2400.0
        wqt = ki - LAG
        if wqt >= 0:
            c += (wqt + 1) * 2 * 0.035
        return c

    def cap(g):
        return 3.4 if g < NT else 2.2

    scopy = nc.scalar.copy
    groups = []  # (deadline_band, [(cost_us, closure)])
    for tcI in range(1, NTC):  # K hi chunks: kt tile 4*tcI first read at
        items = [(0.9, q) for q in kq_quanta(projp, None, tcI, scopy)]
        items.append((0.3, (lambda tcI=tcI: rope(projp, kt, tcI * TCW, TCW))))
        groups.append((tcI * (TCW // 128), items))
    for ti in range(4, NT):
        groups.append((min(ti + LAG, NT - 1),
                       [(0.95, v_filler(projp, ti, scopy))]))
    for m in (1, 2, 3):
        for tcI in range(NTC):
            items = [(0.9, q) for q in kq_quanta(projp, m, tcI, scopy)]
            items.append(
                (0.3, (lambda m=m, tcI=tcI: rope(projp, get_qt(m),
                                                 tcI * TCW, TCW))))
            groups.append((m * NT, items))

    sched = {}
    groups.sort(key=lambda x: x[0])
    g, used = 0, 0.0
    for dl, items in groups:
        for cost, fn in items:
            budget = max(0.4, cap(g) - committed(g))
            if used >= budget:
                g, used = g + 1, 0.0
                assert g < NBANDS, "filler overflow"
            assert g < dl, f"filler deadline violated: band {g} >= {dl}"
            sched.setdefault(g, []).append(fn)
            used += cost

    # ---- attention pairs: bands with woven PV + per-group gpsimd copy ----
    def pair(m):
        qtm = get_qt(m)
        ptsAs, ptsBs = {}, {}
        stg0 = stgp.tile([128, NT // 2, 130], F32, tag="stg0", name="stg0")
        stg1 = stgp.tile([128, NT - NT // 2, 130], F32, tag="stg1",
                         name="stg1")
        state = {"bank": None}

        def emit_group(qt):
            # the two head-regions of one qt sit adjacently (130 wide) in
            # the PSUM bank ring so a single copy drains both
            if qt % 2 == 0:
                state["bank"] = pvp.tile([128, 512], F32, tag="pv",
                                         name="pv")
            bank = state["bank"]
            base = 130 * (qt % 2)
            for h, pts_d in ((0, ptsAs), (1, ptsBs)):
                col = base + 65 * h
                for ki in range(qt + 1):
                    nc.tensor.matmul(
                        bank[:, col:col + 65],
                        pts_d[ki][:, 128 * (qt - ki):128 * (qt - ki) + 128],
                        vts[ki][:, h * 65:(h + 1) * 65],
                        start=(ki == 0), stop=(ki == qt))
            stg = stg0 if qt < NT // 2 else stg1
            j = qt - (0 if qt < NT // 2 else NT // 2)
            cp = nc.scalar.copy if qt % 2 == 0 else nc.vector.tensor_copy
            cp(stg[:, j, :], bank[:, base:base + 130])
            if qt == NT // 2 - 1:
                nc.sync.dma_start(
                    out=out[:, 0:NT // 2, m * 130:(m + 1) * 130],
                    in_=stg0[:])

        for ki in range(NT):
            q0 = 128 * ki
            w = T - q0
            ptsA = ppts.tile([128, w], BF16, tag=f"pa{ki}", name=f"pa{ki}")
            ptsB = ppts.tile([128, w], BF16, tag=f"pb{ki}", name=f"pb{ki}")
            for off in range(0, w, SCW):
                cn = min(SCW, w - off)
                sA = sp.tile([128, SCW], F32, tag="s", name="sA")
                sB = sp.tile([128, SCW], F32, tag="s", name="sB")
                nc.tensor.matmul(
                    sA[:, 0:cn],
                    kt[0:64, ki * 128:(ki + 1) * 128],
                    qtm[0:64, q0 + off:q0 + off + cn],
                    start=True, stop=True)
                nc.tensor.matmul(
                    sB[:, 0:cn],
                    kt[64:128, ki * 128:(ki + 1) * 128],
                    qtm[64:128, q0 + off:q0 + off + cn],
                    start=True, stop=True)
                nc.scalar.activation(ptsA[:, off:off + cn], sA[:, 0:cn],
                                     AF.Exp, scale=SCALE)
                nc.vector.tensor_scalar(
                    out=ptsB[:, off:off + cn].bitcast(I16), in0=sB[:, 0:cn],
                    scalar1=EXP_A, scalar2=EXP_B, op0=ALU.mult, op1=ALU.add)
            # every band starts at its diagonal block
            nc.gpsimd.tensor_tensor(out=ptsA[:, 0:128], in0=ptsA[:, 0:128],
                                    in1=triu[:], op=ALU.mult)
            nc.gpsimd.tensor_tensor(out=ptsB[:, 0:128], in0=ptsB[:, 0:128],
                                    in1=triu[:], op=ALU.mult)
            ptsAs[ki], ptsBs[ki] = ptsA, ptsB

            for f in sched.get(m * NT + ki, ()):
                f()

            if ki - LAG >= 0:
                emit_group(ki - LAG)

        for qt in range(NT - LAG, NT):
            emit_group(qt)
        nc.sync.dma_start(
            out=out[:, NT // 2:NT, m * 130:(m + 1) * 130], in_=stg1[:])

    for m in range(4):
        pair(m)

    ctx.close()


def build_program(T=2048, num_devices=8):
    nc = bacc.Bacc("TRN2", target_bir_lowering=False, debug=False,
                   num_devices=num_devices)
    nd = D // 128
    xT = nc.dram_tensor("xT", (128, nd, T), BF16, kind="ExternalInput").ap()
    wqT = nc.dram_tensor("wqT", (128, nd, 512), BF16, kind="ExternalInput").ap()
    wkT = nc.dram_tensor("wkT", (128, nd, 128), BF16, kind="ExternalInput").ap()
    wvT = nc.dram_tensor("wvT", (128, nd, 128), BF16, kind="ExternalInput").ap()
    cosr = nc.dram_tensor("cosr", (128, T), BF16, kind="ExternalInput").ap()
    sins = nc.dram_tensor("sins", (128, T), BF16, kind="ExternalInput").ap()
    permM = nc.dram_tensor("perm", (128, 128), BF16, kind="ExternalInput").ap()
    # out[p, qt, c]: row qt*128+p of the logical [T, 520] output; columns
    # pair-major: pair m, head h (0=m, 1=m+4) numerator at
    # c in [130m+65h, +64), denominator at 130m+65h+64
    out = nc.dram_tensor("out", (128, T // 128, 520), F32,
                         kind="ExternalOutput").ap()
    with tile.TileContext(nc) as tc:
        _emit_body(tc, (xT, wqT, wkT, wvT, cosr, sins, permM, out), T)
    nc.compile()
    return nc


# ---------------- host side ----------------

def _qperm(j):
    rows = []
    for m in range(4):
        for r in range(128):
            h = m if r < 64 else m + 4
            d = 2 * (r % 32) + (1 if (r % 64) >= 32 else 0)
            rows.append((8 * j + h) * 64 + d)
    return np.array(rows)


def _kperm(j):
    rows = []
    for kv in range(2):
        for r in range(64):
            d = 2 * (r % 32) + (1 if r >= 32 else 0)
            rows.append((2 * j + kv) * 64 + d)
    return np.array(rows)


def _to3d(a):
    """[D, C] -> [128, D//128, C] (partition-major di stacking)."""
    Dd, C = a.shape
    return np.ascontiguousarray(a.reshape(Dd // 128, 128, C).transpose(1, 0, 2))


def _perm_mat():
    p = np.zeros((128, 128), dtype=ml_dtypes.bfloat16)
    for i in range(128):
        j = i + 32 if (i % 64) < 32 else i - 32
        p[i, j] = 1.0
    return p


def make_core_inputs(x, Wq, Wk, Wv, cos, sin):
    """Per-core input dicts (host prep). x: [B,T,D]."""
    bf = ml_dtypes.bfloat16
    B, T, _ = x.shape
    xTb = [_to3d(np.ascontiguousarray(x[b].T).astype(bf)) for b in range(B)]
    cosT = np.ascontiguousarray(cos.T.astype(np.float32))  # [32, T]
    sinT = np.ascontiguousarray(sin.T.astype(np.float32))
    cosr = np.tile(cosT, (4, 1)).astype(bf)
    sgn = np.repeat(np.array([-1.0, 1.0, -1.0, 1.0], np.float32), 32)
    sins = (np.tile(sinT, (4, 1)) * sgn[:, None]).astype(bf)
    perm = _perm_mat()
    maps = []
    for c in range(8):
        b, j = c // 4, c % 4
        maps.append({
            "xT": xTb[b],
            "wqT": _to3d(Wq[_qperm(j)].T.astype(bf)),
            "wkT": _to3d(Wk[_kperm(j)].T.astype(bf)),
            "wvT": _to3d(Wv[128 * j:128 * (j + 1)].T.astype(bf)),
            "cosr": cosr,
            "sins": sins,
            "perm": perm,
        })
    return maps


def core_out_to_full(res_out):
    """res_out: [128, NT, 520] pair-major num/den -> [T, 512] head-major."""
    nt = res_out.shape[1]
    o = np.transpose(res_out, (1, 0, 2)).reshape(nt * 128, 520)
    full = np.empty((nt * 128, 512), np.float32)
    for m in range(4):
        for h in (0, 1):
            base = m * 130 + h * 65
            num = o[:, base:base + 64]
            den = o[:, base + 64:base + 65]
            full[:, (m + 4 * h) * 64:(m + 4 * h) * 64 + 64] = num / den
    return full


_CACHE = {}


def _get_program():
    if "nc" not in _CACHE:
        _CACHE["nc"] = build_program(T=2048, num_devices=8)
    return _CACHE["nc"]


def run_on_hw(in_maps, trace=False):
    nc = _get_program()
    return run_bass_kernel_spmd(nc, in_maps, list(range(8)), trace=trace)


def kernel(x, Wq, Wk, Wv, cos, sin):
    x = np.asarray(x, np.float32)
    Wq = np.asarray(Wq, np.float32)
    Wk = np.asarray(Wk, np.float32)
    Wv = np.asarray(Wv, np.float32)
    cos = np.asarray(cos, np.float32)
    sin = np.asarray(sin, np.float32)
    maps = make_core_inputs(x, Wq, Wk, Wv, cos, sin)
    res = run_on_hw(maps, trace=False)
    B, T = x.shape[0], x.shape[1]
    out = np.empty((B, T, 2048), np.float32)
    for c in range(8):
        b, j = c // 4, c % 4
        out[b, :, 512 * j:512 * (j + 1)] = core_out_to_full(res.results[c]["out"])
    return out


# revision 7
# speedup vs baseline: 1.3390x; 1.0063x over previous
"""GQA kernel v4 for Trainium2 (Bass/Tile), 8 NeuronCores.

Sharding: core c -> batch b=c//4, kv-head pair j=c%4 (kv heads 2j,2j+1,
q heads 8j..8j+7).  Each core computes out[b, :, 512j:512(j+1)] (pair-major
column order + per-head softmax denominator; divided + untangled on host).

v4 structure (trace-driven, from the 288us baseline):
  - ONE attention block per head-pair (q-block = full T): bands ki=0..15,
    each band is S^T(ki) over q cols [128ki, T).  PV accumulation group
    for q-tile qt is woven right after band qt+LAG, so PV matmuls spread
    across the whole pair instead of bursting at block end (the v3
    qb-split left 10-band "deserts" with no PE work -> HAM half-clock).
  - PV regions live in a ring of [128,512] PSUM bank tiles (7 x 65-wide
    regions each, bufs=2); each group's [128,65] numerator+denominator
    is copied to SBUF by GPSIMD right after the group closes (ACT/DVE
    stay free for exp; no end-of-block copy chain).
  - S PSUM ring (tag "s", bufs=4, 512-wide chunks) decouples S matmuls
    from exp consumption.
  - Input DMA split across the two HWDGE queues (sync + scalar),
    priority-ordered so phase A streams just-in-time.
  - PE warm-up matmuls at t=0 (triu x triu) warm the HAM clock gate.
  - RoPE partition swap via a signed-permutation matmul on the PE.
  - Softmax division on the host (numerators + ones-column denominator).
  - exp split per band: head m via ACT (exact), head m+4 via DVE int16
    Schraudolph writing bf16 bit patterns.
"""

import sys

for _p in ("/opt/trn_rl_repo",):
    if _p not in sys.path:
        sys.path.insert(0, _p)

import contextlib

import numpy as np
import ml_dtypes

import concourse.bass as bass
import concourse.tile as tile
from concourse import bacc, mybir
from concourse.bass_utils import run_bass_kernel_spmd
from concourse.masks import make_upper_triangular

BF16 = mybir.dt.bfloat16
F32 = mybir.dt.float32
I16 = mybir.dt.int16
AF = mybir.ActivationFunctionType
ALU = mybir.AluOpType

D = 2048
HS = 64
SCALE = 1.0 / 8.0  # 1/sqrt(HS)

EXP_A = float(128.0 * 1.4426950408889634 * SCALE)
EXP_B = float(127.0 * 128.0 - 366393.0 / 65536.0 - 0.5)

WARMUP_MMS = 44
LAG = 2


def _emit_body(tc, aps, T):
    nc = tc.nc
    NT = T // 128            # k/q tiles == bands per pair
    ND = D // 128            # contraction chunks
    TCW = min(512, T)        # projection t-chunk width
    NTC = T // TCW
    SCW = min(512, T)        # S-chunk width
    NBANDS = 4 * NT

    xT, wqT, wkT, wvT, cosr, sins, permM, out = aps

    ctx = tc._kernel_exitstack = contextlib.ExitStack()

    pers = ctx.enter_context(tc.tile_pool(name="pers", bufs=1))
    rp = ctx.enter_context(tc.tile_pool(name="rope", bufs=2))
    ppts = ctx.enter_context(tc.tile_pool(name="ppts", bufs=1))
    stgp = ctx.enter_context(tc.tile_pool(name="stgp", bufs=1))
    qpool = ctx.enter_context(tc.tile_pool(name="qpool", bufs=2))

    # ---- persistent tiles ----
    xsb = pers.tile([128, ND, T], BF16, tag="xsb")
    wqsb = pers.tile([128, ND, 512], BF16, tag="wqsb")
    wksb = pers.tile([128, ND, 128], BF16, tag="wksb")
    wvsb = pers.tile([128, ND, 128], BF16, tag="wvsb")
    xTs = [xsb[:, di, :] for di in range(ND)]
    wqTs = [wqsb[:, di, :] for di in range(ND)]
    wkTs = [wksb[:, di, :] for di in range(ND)]
    wvTs = [wvsb[:, di, :] for di in range(ND)]
    cosr_t = pers.tile([128, T], BF16, tag="cosr")
    sins_t = pers.tile([128, T], BF16, tag="sins")
    perm_t = pers.tile([128, 128], BF16, tag="perm")

    # ---- input DMA: scalar (ACT HWDGE) carries early tensors + di-hi x
    # halves; sync carries di-lo x halves + the late bulk. ----
    hnd = ND // 2
    qnd = ND // 4
    nc.scalar.dma_start(out=perm_t[:], in_=permM[:, :])
    nc.scalar.dma_start(out=wksb[:], in_=wkT[:, :, :])
    nc.sync.dma_start(out=xsb[:, 0:qnd, 0:TCW], in_=xT[:, 0:qnd, 0:TCW])
    nc.sync.dma_start(out=xsb[:, qnd:hnd, 0:TCW], in_=xT[:, qnd:hnd, 0:TCW])
    nc.scalar.dma_start(out=xsb[:, hnd:3 * qnd, 0:TCW],
                        in_=xT[:, hnd:3 * qnd, 0:TCW])
    nc.scalar.dma_start(out=xsb[:, 3 * qnd:ND, 0:TCW],
                        in_=xT[:, 3 * qnd:ND, 0:TCW])
    nc.scalar.dma_start(out=wqsb[:, :, 0:128], in_=wqT[:, :, 0:128])
    nc.scalar.dma_start(out=cosr_t[:], in_=cosr[:, :])
    nc.scalar.dma_start(out=sins_t[:], in_=sins[:, :])
    for c0 in range(TCW, T, TCW):
        nc.sync.dma_start(out=xsb[:, 0:hnd, c0:c0 + TCW],
                          in_=xT[:, 0:hnd, c0:c0 + TCW])
        nc.scalar.dma_start(out=xsb[:, hnd:ND, c0:c0 + TCW],
                            in_=xT[:, hnd:ND, c0:c0 + TCW])
        if c0 == TCW:
            nc.sync.dma_start(out=wvsb[:], in_=wvT[:, :, :])
    nc.sync.dma_start(out=wqsb[:, :, 128:512], in_=wqT[:, :, 128:512])

    triu = pers.tile([128, 128], BF16, tag="triu")
    make_upper_triangular(nc, triu[:], val=1.0, diag=True)

    # V' tiles: [kv0 64 | one | kv1 64 | one]
    vts = []
    for ti in range(NT):
        v = pers.tile([128, 130], BF16, tag=f"v{ti}", name=f"v{ti}")
        nc.vector.memset(v[:, 64:65], 1.0)
        nc.vector.memset(v[:, 129:130], 1.0)
        vts.append(v)

    kt = pers.tile([128, T], BF16, tag="kt")
    qtile = {}

    def get_qt(m):
        if m not in qtile:
            qtile[m] = qpool.tile([128, T], BF16, tag="qt", name=f"qt{m}")
        return qtile[m]

    def rope(pool, tgt, c0, cw):
        """RoPE on tgt[:, c0:c0+cw]: partition swap via perm matmul on PE,
        then 3 DVE passes (sign is folded into sins host-side)."""
        pp = pool.tile([128, cw], F32, tag="pj", name="ropeps")
        nc.tensor.matmul(pp[:], perm_t[:], tgt[:, c0:c0 + cw],
                         start=True, stop=True)
        swp = rp.tile([128, cw], BF16, tag="swp", name="swp")
        nc.vector.tensor_tensor(out=swp[:], in0=pp[:],
                                in1=sins_t[:, c0:c0 + cw], op=ALU.mult)
        tmp = rp.tile([128, cw], BF16, tag="tmp", name="tmp")
        nc.vector.tensor_tensor(out=tmp[:], in0=tgt[:, c0:c0 + cw],
                                in1=cosr_t[:, c0:c0 + cw], op=ALU.mult)
        nc.vector.tensor_tensor(out=tgt[:, c0:c0 + cw], in0=tmp[:], in1=swp[:],
                                op=ALU.add)

    def kq_quanta(pool, m, tcI, copy_eng):
        """4 quanta for one t-chunk of the K (m is None) or Qm projection."""
        state = {}

        def quantum(k):
            if k == 0:
                state["ps"] = pool.tile([128, TCW], F32, tag="pj", name="pjq")
            ps = state["ps"]
            for di in range(k * 4, k * 4 + 4):
                lhs = wkTs[di][:] if m is None else \
                    wqTs[di][:, m * 128:(m + 1) * 128]
                nc.tensor.matmul(
                    ps[:], lhs, xTs[di][:, tcI * TCW:(tcI + 1) * TCW],
                    start=(di == 0), stop=(di == ND - 1))
            if k == 3:
                tgt = kt if m is None else get_qt(m)
                copy_eng(tgt[:, tcI * TCW:(tcI + 1) * TCW], ps[:])

        return [lambda kk=k: quantum(kk) for k in range(4)]

    def v_filler(pool, ti, copy_eng):
        def f():
            psf = pool.tile([128, TCW], F32, tag="pj", name="pjv")
            ps = psf[:, 0:128]
            for di in range(ND):
                nc.tensor.matmul(
                    ps, xTs[di][:, ti * 128:(ti + 1) * 128], wvTs[di][:],
                    start=(di == 0), stop=(di == ND - 1))
            copy_eng(vts[ti][:, 0:64], ps[:, 0:64])
            copy_eng(vts[ti][:, 65:129], ps[:, 64:128])
        return f

    # ---- phase A: PE warm-up, K c0, all of Q0, V0-3 (+ropes), JIT with
    # the x chunk arrival order.  Dummy triu matmuls (no data deps) pad
    # the DMA-wait points so the HAM clock gate never sees a long idle. ----
    with tc.tile_pool(name="ppe", bufs=4, space="PSUM") as ppe:
        warm = ppe.tile([128, 128], F32, tag="warm", bufs=1, name="warm")

        def dummyA(n):
            for _ in range(n):
                nc.tensor.matmul(warm[:], triu[:], triu[:],
                                 start=True, stop=True)

        dummyA(WARMUP_MMS)
        vcopy = nc.vector.tensor_copy
        seq = [("k", 0), ("q", 0)]
        for tcI in range(1, NTC):
            seq.append(("w", 10))
            if tcI == 2:
                seq += [("v", 0), ("v", 1)]
            if tcI == NTC - 1:
                seq += [("v", 2), ("v", 3)]
            seq.append(("q", tcI))
        for kind, i in seq:
            if kind == "k":
                for q in kq_quanta(ppe, None, i, vcopy):
                    q()
                rope(ppe, kt, i * TCW, TCW)
            elif kind == "q":
                for q in kq_quanta(ppe, 0, i, vcopy):
                    q()
                rope(ppe, get_qt(0), i * TCW, TCW)
            elif kind == "w":
                dummyA(i)
            else:
                v_filler(ppe, i, vcopy)()

    # ---- attention pools (8 PSUM banks: 4 + 2 + 2) ----
    sp = ctx.enter_context(tc.tile_pool(name="spsum", bufs=5, space="PSUM"))
    pvp = ctx.enter_context(tc.tile_pool(name="pvp", bufs=1, space="PSUM"))
    projp = ctx.enter_context(tc.tile_pool(name="projp", bufs=2, space="PSUM"))

    # ---- filler schedule: EDF into global bands with per-band budgets ----
    def committed(g):
        ki = g % NT
        c = (T - 128 * ki) / 2400.0
        wqt = ki - LAG
        if wqt >= 0:
            c += (wqt + 1) * 2 * 0.035
        return c

    def cap(g):
        return 3.4 if g < NT else 2.2

    scopy = nc.scalar.copy
    groups = []  # (deadline_band, [(cost_us, closure)])
    for tcI in range(1, NTC):  # K hi chunks: kt tile 4*tcI first read at
        items = [(0.9, q) for q in kq_quanta(projp, None, tcI, scopy)]
        items.append((0.3, (lambda tcI=tcI: rope(projp, kt, tcI * TCW, TCW))))
        groups.append((tcI * (TCW // 128), items))
    for ti in range(4, NT):
        groups.append((min(ti + LAG, NT - 1),
                       [(0.95, v_filler(projp, ti, scopy))]))
    for m in (1, 2, 3):
        for tcI in range(NTC):
            items = [(0.9, q) for q in kq_quanta(projp, m, tcI, scopy)]
            items.append(
                (0.3, (lambda m=m, tcI=tcI: rope(projp, get_qt(m),
                                                 tcI * TCW, TCW))))
            groups.append((m * NT, items))

    sched = {}
    sched_cost = {}
    groups.sort(key=lambda x: x[0])
    g, used = 0, 0.0
    for dl, items in groups:
        for cost, fn in items:
            budget = max(0.4, cap(g) - committed(g))
            if used >= budget:
                g, used = g + 1, 0.0
                assert g < NBANDS, "filler overflow"
            assert g < dl, f"filler deadline violated: band {g} >= {dl}"
            sched.setdefault(g, []).append(fn)
            sched_cost[g] = sched_cost.get(g, 0.0) + cost
            used += cost

    # ---- attention pairs: bands with woven PV + per-group gpsimd copy ----
    def pair(m):
        qtm = get_qt(m)
        ptsAs, ptsBs = {}, {}
        stg0 = stgp.tile([128, NT // 2, 130], F32, tag="stg0", name="stg0")
        stg1 = stgp.tile([128, NT - NT // 2, 130], F32, tag="stg1",
                         name="stg1")
        state = {"bank": None}

        def emit_group(qt):
            # the two head-regions of one qt sit adjacently (130 wide) in
            # the PSUM bank ring so a single copy drains both
            if qt % 2 == 0:
                state["bank"] = pvp.tile([128, 512], F32, tag="pv",
                                         name="pv")
            bank = state["bank"]
            base = 130 * (qt % 2)
            for h, pts_d in ((0, ptsAs), (1, ptsBs)):
                col = base + 65 * h
                for ki in range(qt + 1):
                    nc.tensor.matmul(
                        bank[:, col:col + 65],
                        pts_d[ki][:, 128 * (qt - ki):128 * (qt - ki) + 128],
                        vts[ki][:, h * 65:(h + 1) * 65],
                        start=(ki == 0), stop=(ki == qt))
            stg = stg0 if qt < NT // 2 else stg1
            j = qt - (0 if qt < NT // 2 else NT // 2)
            cp = nc.scalar.copy if qt % 2 == 0 else nc.vector.tensor_copy
            cp(stg[:, j, :], bank[:, base:base + 130])
            if qt == NT // 2 - 1:
                nc.sync.dma_start(
                    out=out[:, 0:NT // 2, m * 130:(m + 1) * 130],
                    in_=stg0[:])
            elif qt == (3 * NT) // 4 - 1:
                nc.sync.dma_start(
                    out=out[:, NT // 2:(3 * NT) // 4,
                            m * 130:(m + 1) * 130],
                    in_=stg1[:, 0:(3 * NT) // 4 - NT // 2, :])

        for ki in range(NT):
            q0 = 128 * ki
            w = T - q0
            ptsA = ppts.tile([128, w], BF16, tag=f"pa{ki}", name=f"pa{ki}")
            ptsB = ppts.tile([128, w], BF16, tag=f"pb{ki}", name=f"pb{ki}")
            for off in range(0, w, SCW):
                cn = min(SCW, w - off)
                sA = sp.tile([128, SCW], F32, tag="s", name="sA")
                sB = sp.tile([128, SCW], F32, tag="s", name="sB")
                nc.tensor.matmul(
                    sA[:, 0:cn],
                    kt[0:64, ki * 128:(ki + 1) * 128],
                    qtm[0:64, q0 + off:q0 + off + cn],
                    start=True, stop=True)
                nc.tensor.matmul(
                    sB[:, 0:cn],
                    kt[64:128, ki * 128:(ki + 1) * 128],
                    qtm[64:128, q0 + off:q0 + off + cn],
                    start=True, stop=True)
                nc.scalar.activation(ptsA[:, off:off + cn], sA[:, 0:cn],
                                     AF.Exp, scale=SCALE)
                nc.vector.tensor_scalar(
                    out=ptsB[:, off:off + cn].bitcast(I16), in0=sB[:, 0:cn],
                    scalar1=EXP_A, scalar2=EXP_B, op0=ALU.mult, op1=ALU.add)
            # every band starts at its diagonal block
            nc.gpsimd.tensor_tensor(out=ptsA[:, 0:128], in0=ptsA[:, 0:128],
                                    in1=triu[:], op=ALU.mult)
            nc.gpsimd.tensor_tensor(out=ptsB[:, 0:128], in0=ptsB[:, 0:128],
                                    in1=triu[:], op=ALU.mult)
            ptsAs[ki], ptsBs[ki] = ptsA, ptsB

            for f in sched.get(m * NT + ki, ()):
                f()

            if ki - LAG >= 0:
                emit_group(ki - LAG)

            # keep-warm: pad underloaded bands with dep-free matmuls into
            # the unused tail of the current PV bank so the HAM clock
            # gate never down-clocks the PE
            if state["bank"] is not None:
                load = committed(m * NT + ki) + sched_cost.get(m * NT + ki,
                                                               0.0)
                ndum = min(6, max(0, int((1.5 - load) / 0.107)))
                for _ in range(ndum):
                    nc.tensor.matmul(state["bank"][:, 260:512],
                                     triu[:], kt[:, 0:252],
                                     start=True, stop=True)

        for qt in range(NT - LAG, NT):
            emit_group(qt)
        nc.sync.dma_start(
            out=out[:, (3 * NT) // 4:NT, m * 130:(m + 1) * 130],
            in_=stg1[:, (3 * NT) // 4 - NT // 2:, :])

    for m in range(4):
        pair(m)

    ctx.close()


def build_program(T=2048, num_devices=8):
    nc = bacc.Bacc("TRN2", target_bir_lowering=False, debug=False,
                   num_devices=num_devices)
    nd = D // 128
    xT = nc.dram_tensor("xT", (128, nd, T), BF16, kind="ExternalInput").ap()
    wqT = nc.dram_tensor("wqT", (128, nd, 512), BF16, kind="ExternalInput").ap()
    wkT = nc.dram_tensor("wkT", (128, nd, 128), BF16, kind="ExternalInput").ap()
    wvT = nc.dram_tensor("wvT", (128, nd, 128), BF16, kind="ExternalInput").ap()
    cosr = nc.dram_tensor("cosr", (128, T), BF16, kind="ExternalInput").ap()
    sins = nc.dram_tensor("sins", (128, T), BF16, kind="ExternalInput").ap()
    permM = nc.dram_tensor("perm", (128, 128), BF16, kind="ExternalInput").ap()
    # out[p, qt, c]: row qt*128+p of the logical [T, 520] output; columns
    # pair-major: pair m, head h (0=m, 1=m+4) numerator at
    # c in [130m+65h, +64), denominator at 130m+65h+64
    out = nc.dram_tensor("out", (128, T // 128, 520), F32,
                         kind="ExternalOutput").ap()
    with tile.TileContext(nc) as tc:
        _emit_body(tc, (xT, wqT, wkT, wvT, cosr, sins, permM, out), T)
    nc.compile()
    return nc


# ---------------- host side ----------------

def _qperm(j):
    rows = []
    for m in range(4):
        for r in range(128):
            h = m if r < 64 else m + 4
            d = 2 * (r % 32) + (1 if (r % 64) >= 32 else 0)
            rows.append((8 * j + h) * 64 + d)
    return np.array(rows)


def _kperm(j):
    rows = []
    for kv in range(2):
        for r in range(64):
            d = 2 * (r % 32) + (1 if r >= 32 else 0)
            rows.append((2 * j + kv) * 64 + d)
    return np.array(rows)


def _to3d(a):
    """[D, C] -> [128, D//128, C] (partition-major di stacking)."""
    Dd, C = a.shape
    return np.ascontiguousarray(a.reshape(Dd // 128, 128, C).transpose(1, 0, 2))


def _perm_mat():
    p = np.zeros((128, 128), dtype=ml_dtypes.bfloat16)
    for i in range(128):
        j = i + 32 if (i % 64) < 32 else i - 32
        p[i, j] = 1.0
    return p


def make_core_inputs(x, Wq, Wk, Wv, cos, sin):
    """Per-core input dicts (host prep). x: [B,T,D]."""
    bf = ml_dtypes.bfloat16
    B, T, _ = x.shape
    xTb = [_to3d(np.ascontiguousarray(x[b].T).astype(bf)) for b in range(B)]
    cosT = np.ascontiguousarray(cos.T.astype(np.float32))  # [32, T]
    sinT = np.ascontiguousarray(sin.T.astype(np.float32))
    cosr = np.tile(cosT, (4, 1)).astype(bf)
    sgn = np.repeat(np.array([-1.0, 1.0, -1.0, 1.0], np.float32), 32)
    sins = (np.tile(sinT, (4, 1)) * sgn[:, None]).astype(bf)
    perm = _perm_mat()
    maps = []
    for c in range(8):
        b, j = c // 4, c % 4
        maps.append({
            "xT": xTb[b],
            "wqT": _to3d(Wq[_qperm(j)].T.astype(bf)),
            "wkT": _to3d(Wk[_kperm(j)].T.astype(bf)),
            "wvT": _to3d(Wv[128 * j:128 * (j + 1)].T.astype(bf)),
            "cosr": cosr,
            "sins": sins,
            "perm": perm,
        })
    return maps


def core_out_to_full(res_out):
    """res_out: [128, NT, 520] pair-major num/den -> [T, 512] head-major."""
    nt = res_out.shape[1]
    o = np.transpose(res_out, (1, 0, 2)).reshape(nt * 128, 520)
    full = np.empty((nt * 128, 512), np.float32)
    for m in range(4):
        for h in (0, 1):
            base = m * 130 + h * 65
            num = o[:, base:base + 64]
            den = o[:, base + 64:base + 65]
            full[:, (m + 4 * h) * 64:(m + 4 * h) * 64 + 64] = num / den
    return full


_CACHE = {}


def _get_program():
    if "nc" not in _CACHE:
        _CACHE["nc"] = build_program(T=2048, num_devices=8)
    return _CACHE["nc"]


def run_on_hw(in_maps, trace=False):
    nc = _get_program()
    return run_bass_kernel_spmd(nc, in_maps, list(range(8)), trace=trace)


def kernel(x, Wq, Wk, Wv, cos, sin):
    x = np.asarray(x, np.float32)
    Wq = np.asarray(Wq, np.float32)
    Wk = np.asarray(Wk, np.float32)
    Wv = np.asarray(Wv, np.float32)
    cos = np.asarray(cos, np.float32)
    sin = np.asarray(sin, np.float32)
    maps = make_core_inputs(x, Wq, Wk, Wv, cos, sin)
    res = run_on_hw(maps, trace=False)
    B, T = x.shape[0], x.shape[1]
    out = np.empty((B, T, 2048), np.float32)
    for c in range(8):
        b, j = c // 4, c % 4
        out[b, :, 512 * j:512 * (j + 1)] = core_out_to_full(res.results[c]["out"])
    return out
